# revision 1
# baseline (speedup 1.0000x reference)
"""Fused single-head attention (QKV projection + softmax(QK^T)V) on 8 trn2 cores.

Problem (hardcoded): x [4, 4096, 768] f32, W_qkv [768, 2304] f32, b_qkv [2304] f32.
  qkv = x @ W_qkv + b_qkv ; q,k,v = split(qkv, 3)
  out = softmax(q k^T / sqrt(768)) v          -> [4, 4096, 768] f32

Sharding: batch (4) x key-halves (2) -> 8 cores, no cross-core traffic.
Each core gets one batch's x (pre-transposed on host to xT [768, 4096] fp16,
with the key half it owns rotated into columns [0, 2048)), projects q for
all 4096 queries but k/v only for its 2048 keys, and computes PARTIAL
attention sums over those keys:
  outT_partial [768, 4096] = sum_j exp(q k_j^T / sqrt(H)) v_j   (fp32)
  den_partial  [4096]      = sum_j exp(q k_j^T / sqrt(H))
The host combines the two partials of each pair: (o0 + o1) / (d0 + d1).
No max-subtraction is needed: scores here are O(1), exp is safe in
fp16/fp32, and both partials use the same (absent) shift so the combine is
exact softmax.

On-chip layout ("transposed flash attention"):
  - qkv computed in head-major layout qT/kT [H, n] via lhsT=W, rhs=xT; v in
    [n, H] via lhsT=xT, rhs=W (no on-chip transposes anywhere).
  - scores computed transposed: sT[j, i] = (kT j-tile).T @ qT -> PSUM,
    exp via ScalarE (scale folded in), p stored fp16.
  - denominator: S = sum_jt p_jt accumulated on VectorE (fp32, then cast to
    fp16) and shipped to the host, which finishes the partition-dim sum.
  - outT[h, i] += (v j-tile).T @ p accumulated over j in PSUM.
  - PV matmuls run one j-tile behind QK (software pipeline) so the PE
    always has work covering the exp; PSUM evacuations of i-block N are
    deferred into i-block N+1 and split across VectorE/ScalarE.
PSUM budget (8 banks): scores 2 + out accumulators 6.
"""

import math
from contextlib import ExitStack
from functools import lru_cache

import numpy as np

import concourse.bacc as bacc
import concourse.bass as bass
import concourse.tile as tile
from concourse import mybir
from concourse.bass_utils import run_bass_kernel_spmd

B, N, C = 4, 4096, 768
H = 768          # head dim (== C)
H3 = 3 * H
NCORES = 8
NK = N // 2      # keys per core
DT = mybir.dt.float16
F32 = mybir.dt.float32
SCALE = 1.0 / math.sqrt(H)

CT = C // 128    # 6 contraction tiles (c)
HT = H // 128    # 6 head tiles (h)
JT = NK // 128   # 16 key tiles (j) per core
RB = 8           # r-blocks of 512 over the 4096 rows
RBS = N // RB    # 512
KRB = RB // 2    # r-blocks that contain this core's keys (first 4)
IB = 8           # i-blocks of 512 over all 4096 queries
IBS = N // IB    # 512


def build_program():
    nc = bacc.Bacc(
        "TRN2",
        target_bir_lowering=False,
        debug=False,
        enable_asserts=False,
        num_devices=NCORES,
    )
    xT_d = nc.dram_tensor("xT", [C, N], DT, kind="ExternalInput").ap()
    w_d = nc.dram_tensor("w", [C, H3], DT, kind="ExternalInput").ap()
    bqk_d = nc.dram_tensor("bqk", [128, 2 * HT], F32, kind="ExternalInput").ap()
    bv_d = nc.dram_tensor("bv", [128, H], F32, kind="ExternalInput").ap()
    outT_d = nc.dram_tensor("outT", [H, N], F32, kind="ExternalOutput").ap()
    # per-partition partial softmax denominators; host sums over axis 1
    den_d = nc.dram_tensor("den", [IB, 128, IBS], DT, kind="ExternalOutput").ap()

    with tile.TileContext(nc) as tc:
        with ExitStack() as ctx:
            persist = ctx.enter_context(tc.tile_pool(name="persist", bufs=1))

            kT = [persist.tile([128, NK], DT, tag=f"kT{t}", name=f"kT{t}")
                  for t in range(HT)]
            qT = [persist.tile([128, N], DT, tag=f"qT{t}", name=f"qT{t}")
                  for t in range(HT)]
            vv = [persist.tile([128, H], DT, tag=f"v{t}", name=f"v{t}")
                  for t in range(JT)]
            bqk = persist.tile([128, 2 * HT], F32, tag="bqk")
            bvb = persist.tile([128, H], F32, tag="bvb")

            # ---- Phase 1: QKV projection ----
            with tc.tile_pool(name="wpool", bufs=1) as wpool, \
                 tc.tile_pool(name="xpool", bufs=3 * CT) as xpool, \
                 tc.tile_pool(name="pj", bufs=4, space="PSUM") as pj, \
                 tc.tile_pool(name="pv", bufs=2, space="PSUM") as pv:

                ws = [wpool.tile([128, H3], DT, tag=f"w{t}", name=f"w{t}")
                      for t in range(CT)]

                def load_xt(rb):
                    r0 = rb * RBS
                    tiles = []
                    for ct in range(CT):
                        t = xpool.tile([128, RBS], DT, tag="xt", name=f"xt{rb}_{ct}")
                        nc.sync.dma_start(
                            out=t, in_=xT_d[ct * 128:(ct + 1) * 128, r0:r0 + RBS])
                        tiles.append(t)
                    return tiles

                # DMA issue order = need order: first r-block's x, then the
                # k-projection h-tile-0 columns of W, then the small biases,
                # then the rest of W.
                # interleave so the ct=0 matmul's two inputs are the first
                # two DMAs in the queue, ct=1's the next two, ...
                xts = [None] * RB
                xt0 = []
                for ct in range(CT):
                    nc.sync.dma_start(
                        out=ws[ct][:, H: H + 128],
                        in_=w_d[ct * 128:(ct + 1) * 128, H: H + 128])
                    t = xpool.tile([128, RBS], DT, tag="xt", name=f"xt0_{ct}")
                    nc.sync.dma_start(out=t, in_=xT_d[ct * 128:(ct + 1) * 128, 0:RBS])
                    xt0.append(t)
                xts[0] = xt0
                for ht in range(1, HT):
                    for ct in range(CT):
                        nc.sync.dma_start(
                            out=ws[ct][:, H + ht * 128: H + (ht + 1) * 128],
                            in_=w_d[ct * 128:(ct + 1) * 128,
                                    H + ht * 128: H + (ht + 1) * 128])
                    if ht == 1:
                        nc.sync.dma_start(out=bqk, in_=bqk_d)
                for ct in range(CT):
                    nc.sync.dma_start(out=ws[ct][:, 0:H],
                                      in_=w_d[ct * 128:(ct + 1) * 128, 0:H])
                    nc.sync.dma_start(out=ws[ct][:, 2 * H:H3],
                                      in_=w_d[ct * 128:(ct + 1) * 128, 2 * H:H3])
                nc.sync.dma_start(out=bvb, in_=bv_d)

                # PE warm-up: ~3.4us of junk matmuls (no DMA deps) so the
                # HAM clock-gate reaches full rate while the first x/W DMAs
                # are still in flight.
                warm_l = xpool.tile([128, 128], DT, tag="warml", name="warml")
                warm_r = xpool.tile([128, 512], DT, tag="warmr", name="warmr")
                nc.vector.memset(warm_l, 0.0)
                nc.vector.memset(warm_r, 0.0)
                for i in range(16):
                    wp = pj.tile([128, RBS], F32, tag="pj", name=f"warm{i}")
                    nc.tensor.matmul(wp, warm_l, warm_r, start=True, stop=True)

                for rb in range(RB):
                    r0 = rb * RBS
                    if rb + 1 < RB:
                        xts[rb + 1] = load_xt(rb + 1)
                    xt = xts[rb]

                    projs = [(0, qT, r0)]          # q: every r-block
                    if rb < KRB:
                        projs.insert(0, (H, kT, r0))   # k: first half only
                    for (wofs, dst, c0) in projs:
                        for ht in range(HT):
                            ps = pj.tile([128, RBS], F32, tag="pj")
                            for ct in range(CT):
                                nc.tensor.matmul(
                                    ps,
                                    ws[ct][:, wofs + ht * 128: wofs + (ht + 1) * 128],
                                    xt[ct],
                                    start=(ct == 0), stop=(ct == CT - 1),
                                )
                            bcol = (0 if wofs == 0 else HT) + ht
                            nc.scalar.activation(
                                out=dst[ht][:, c0:c0 + RBS],
                                in_=ps,
                                func=mybir.ActivationFunctionType.Identity,
                                bias=bqk[:, bcol:bcol + 1],
                            )

                    if rb < KRB:
                        for j in range(RBS // 128):
                            jt = rb * (RBS // 128) + j
                            ps = pv.tile([128, H], F32, tag="pv")
                            for ct in range(CT):
                                xs = xt[ct][:, j * 128:(j + 1) * 128]
                                nc.tensor.matmul(
                                    ps[:, 0:512], xs, ws[ct][:, 2 * H: 2 * H + 512],
                                    start=(ct == 0), stop=(ct == CT - 1))
                                nc.tensor.matmul(
                                    ps[:, 512:H], xs, ws[ct][:, 2 * H + 512: 3 * H],
                                    start=(ct == 0), stop=(ct == CT - 1))
                            nc.vector.tensor_add(vv[jt], ps, bvb)

            # ---- Phase 2: attention (partial sums over this core's keys) ----
            with tc.tile_pool(name="ppool", bufs=1) as ppool, \
                 tc.tile_pool(name="opool", bufs=8) as opool, \
                 tc.tile_pool(name="spool", bufs=2) as spool, \
                 tc.tile_pool(name="ps_s", bufs=2, space="PSUM") as ps_s, \
                 tc.tile_pool(name="ps_o", bufs=6, space="PSUM") as ps_o:
                p_t = [ppool.tile([128, IBS], DT, tag=f"p{t}", name=f"p{t}")
                       for t in range(JT)]

                pending = []   # deferred work, flushed between PE groups

                def flush():
                    while pending:
                        pending.pop(0)()

                def emit_pv(og, jt, i0):
                    def go():
                        for ht in range(HT):
                            nc.tensor.matmul(
                                og[ht],
                                vv[jt][:, ht * 128:(ht + 1) * 128],
                                p_t[jt],
                                start=(jt == 0), stop=(jt == JT - 1),
                            )
                    pending.append(go)

                def emit_den_and_evac(og, S16, ib, i0):
                    def go():
                        nc.sync.dma_start(out=den_d[ib], in_=S16)
                        for ht in range(HT):
                            ot = opool.tile([128, IBS], F32, tag="ot",
                                            name=f"ot{i0}_{ht}")
                            if ht % 2 == 0:
                                nc.vector.tensor_copy(out=ot, in_=og[ht])
                                dma = nc.sync.dma_start
                            else:
                                nc.scalar.activation(
                                    out=ot, in_=og[ht],
                                    func=mybir.ActivationFunctionType.Copy)
                                dma = nc.scalar.dma_start
                            dma(out=outT_d[ht * 128:(ht + 1) * 128,
                                           i0:i0 + IBS],
                                in_=ot)
                    pending.append(go)

                for ib in range(IB):
                    i0 = ib * IBS
                    og = [ps_o.tile([128, IBS], F32, tag="o", name=f"o{ib}_{g}")
                          for g in range(HT)]
                    Sf = spool.tile([128, IBS], F32, tag="Sf", name=f"Sf{ib}")
                    for jt in range(JT):
                        sps = ps_s.tile([128, IBS], F32, tag="s")
                        for ht in range(HT):
                            nc.tensor.matmul(
                                sps,
                                kT[ht][:, jt * 128:(jt + 1) * 128],
                                qT[ht][:, i0:i0 + IBS],
                                start=(ht == 0), stop=(ht == HT - 1),
                            )
                        flush()
                        nc.scalar.activation(
                            out=p_t[jt], in_=sps,
                            func=mybir.ActivationFunctionType.Exp,
                            scale=SCALE,
                        )
                        if jt == 0:
                            nc.vector.tensor_copy(out=Sf, in_=p_t[jt])
                        else:
                            nc.vector.tensor_add(Sf, Sf, p_t[jt])
                        if not (ib == IB - 1 and jt == JT - 1):
                            emit_pv(og, jt, i0)
                    S16 = spool.tile([128, IBS], DT, tag="S16", name=f"S16{ib}")
                    nc.vector.tensor_copy(out=S16, in_=Sf)
                    if ib < IB - 1:
                        emit_den_and_evac(og, S16, ib, i0)
                    else:
                        # eager epilogue: interleave the final j-tile's PV
                        # matmuls with per-h-tile evacuation so the output
                        # DMAs start as early as possible.
                        def epilogue(og=og, S16=S16, ib=ib, i0=i0):
                            nc.sync.dma_start(out=den_d[ib], in_=S16)
                            for ht in range(HT):
                                nc.tensor.matmul(
                                    og[ht],
                                    vv[JT - 1][:, ht * 128:(ht + 1) * 128],
                                    p_t[JT - 1],
                                    start=False, stop=True,
                                )
                                ot = opool.tile([128, IBS], F32, tag="ot",
                                                name=f"ot{i0}_{ht}")
                                if ht % 2 == 0:
                                    nc.vector.tensor_copy(out=ot, in_=og[ht])
                                    dma = nc.sync.dma_start
                                else:
                                    nc.scalar.activation(
                                        out=ot, in_=og[ht],
                                        func=mybir.ActivationFunctionType.Copy)
                                    dma = nc.scalar.dma_start
                                dma(out=outT_d[ht * 128:(ht + 1) * 128,
                                               i0:i0 + IBS],
                                    in_=ot)
                        pending.append(epilogue)
                flush()
    nc.compile()
    return nc


@lru_cache(maxsize=1)
def _cached_program():
    return build_program()


def _prep_in_maps(x, W_qkv, b_qkv):
    x = np.asarray(x, dtype=np.float32)
    W_qkv = np.asarray(W_qkv, dtype=np.float32)
    b_qkv = np.asarray(b_qkv, dtype=np.float32)
    w16 = W_qkv.astype(np.float16)
    bq = b_qkv[0:H].astype(np.float32).reshape(HT, 128).T    # [128, HT]
    bk = b_qkv[H:2 * H].astype(np.float32).reshape(HT, 128).T
    bqk = np.ascontiguousarray(np.concatenate([bq, bk], axis=1))  # [128, 2*HT]
    bv = np.ascontiguousarray(
        np.broadcast_to(b_qkv[2 * H:3 * H].astype(np.float32), (128, H)))

    in_maps = []
    for core in range(NCORES):
        b, kh = core // 2, core % 2
        xb = x[b]  # [N, C] f32
        if kh == 1:
            # Rotate so this core's key rows occupy rows [0, NK). Queries are
            # also rotated; the host rotates this core's outputs back.
            xb = np.concatenate([xb[NK:], xb[:NK]], axis=0)
        xT = np.ascontiguousarray(xb.T).astype(np.float16)
        in_maps.append({"xT": xT, "w": w16, "bqk": bqk, "bv": bv})
    return in_maps


def _combine(results):
    out = np.empty((B, N, C), dtype=np.float32)
    for b in range(B):
        o0 = results[2 * b]["outT"]              # [H, N]
        d0 = results[2 * b]["den"].astype(np.float32).sum(axis=1).reshape(N)
        o1 = results[2 * b + 1]["outT"]
        d1 = results[2 * b + 1]["den"].astype(np.float32).sum(axis=1).reshape(N)
        # core (2b+1) worked in rotated query order; rotate back
        o1 = np.concatenate([o1[:, NK:], o1[:, :NK]], axis=1)
        d1 = np.concatenate([d1[NK:], d1[:NK]])
        out[b] = ((o0 + o1) / (d0 + d1)).T
    return out


def kernel(x, W_qkv, b_qkv):
    nc = _cached_program()
    in_maps = _prep_in_maps(x, W_qkv, b_qkv)
    res = run_bass_kernel_spmd(nc, in_maps, core_ids=list(range(NCORES)))
    return _combine(res.results)



# revision 2
# speedup vs baseline: 1.1281x; 1.1281x over previous
"""Fused single-head attention (QKV proj + softmax(QK^T)V) on 8 trn2 cores.

Problem (hardcoded): x [4, 4096, 768] f32, W_qkv [768, 2304] f32, b_qkv
[2304] f32:
  qkv = x @ W_qkv + b_qkv ; q,k,v = split(qkv, 3)
  out = softmax(q k^T / sqrt(768)) v          -> [4, 4096, 768] f32

Sharding: batch (4) x key-halves (2) -> 8 cores, no cross-core traffic.
Each core computes PARTIAL attention sums over its 2048 keys; the host
combines pairs: (o0 + o1) / (d0 + d1). No max-subtraction (scores are
O(1)).

v2 math (vs the fp16 v1): the score chain runs through fp8-e4m3
DoubleRow matmuls at 2x PE rate:
  - k-projection is eliminated: scores = x M x^T * SCALE with
    M = W_q W_k^T precomputed on host (fp16, pre-scaled by 32).
  - q-bias cancels in softmax; k-bias folds into a per-key additive
    score bias w_j = (x @ (W_k b_q) + b_q.b_k) * SCALE computed on the
    host and applied inside the exp activation (per-partition bias).
  - v-bias is added by the host after the combine.
  - t = x @ (32 M) computed on-device in fp16 (f32 accum), stored fp8.
  - scores psum = (32 t) @ x8^T via 3 fp8 DoubleRow matmuls (256-deep
    contraction each); exp scale folds the 1/32 and 1/sqrt(H).
  - PV and v-projection stay fp16: fp8 there fails the 2e-2 gate
    (v/p quantization error passes straight to the output).
Measured-on-hw notes: DoubleRow sustains full rate ONLY when both
operands are sliced from single big SBUF tiles (separate small tiles
rotating per-matmul serialize ~2.3x); f32->fp8 evac on ScalarE/DVE is
exact round-to-nearest (required: truncation would double the error).

Layouts (all per-partition-major, single big tiles):
  xx8 [128, 6, 2048] fp8   x^T for this core's keys, c-tile-structured
  tt8 [128, 6, 4096] fp8   (32 t)^T, written by t-proj evac
  vv  [128, 16, 768] fp16  v tiles (j-tile-structured)
  p16 [128, 16, 512] fp16  exp(scores) for the current i-block
PSUM: scores 2 banks + out accumulators 6 banks.
"""

import math
from contextlib import ExitStack
from functools import lru_cache

import ml_dtypes
import numpy as np

import concourse.bacc as bacc
import concourse.bass as bass
import concourse.tile as tile
from concourse import mybir
from concourse.bass_utils import run_bass_kernel_spmd

B, N, C = 4, 4096, 768
H = 768
NCORES = 8
NK = N // 2      # keys per core
DT = mybir.dt.float16
DT8 = mybir.dt.float8e4
F32 = mybir.dt.float32
SCALE = 1.0 / math.sqrt(H)
TSCALE = 32.0    # host pre-scales M by this; exp folds 1/TSCALE back out
DRMODE = mybir.MatmulPerfMode.DoubleRow

CT = C // 128    # 6 contraction tiles
HT = H // 128    # 6 head tiles
JT = NK // 128   # 16 key tiles per core
RB = 8           # r-blocks of 512 over the 4096 rows
RBS = N // RB    # 512
KRB = RB // 2    # r-blocks containing this core's keys (first 4)
IB = 8
IBS = N // IB    # 512
NP8 = ml_dtypes.float8_e4m3


def build_program():
    nc = bacc.Bacc(
        "TRN2",
        target_bir_lowering=False,
        debug=False,
        enable_asserts=False,
        num_devices=NCORES,
    )
    xT16_d = nc.dram_tensor("xT16", [128, CT, N], DT, kind="ExternalInput").ap()
    xx8_d = nc.dram_tensor("xx8", [128, CT, NK], DT8, kind="ExternalInput").ap()
    m16_d = nc.dram_tensor("m16", [128, CT, H], DT, kind="ExternalInput").ap()
    wv16_d = nc.dram_tensor("wv16", [128, CT, H], DT, kind="ExternalInput").ap()
    wj_d = nc.dram_tensor("wj", [128, JT], F32, kind="ExternalInput").ap()
    outT_d = nc.dram_tensor("outT", [H, N], DT, kind="ExternalOutput").ap()
    # per-partition partial softmax denominators; host sums over axis 1
    den_d = nc.dram_tensor("den", [IB, 128, IBS], DT, kind="ExternalOutput").ap()

    with tile.TileContext(nc) as tc:
        with ExitStack() as ctx:
            persist = ctx.enter_context(tc.tile_pool(name="persist", bufs=1))

            xx8 = persist.tile([128, CT, NK], DT8, tag="xx8")
            tt8 = persist.tile([128, CT, N], DT8, tag="tt8")
            vv = persist.tile([128, JT, H], DT, tag="vv")
            p16 = persist.tile([128, JT, IBS], DT, tag="p16")
            wj = persist.tile([128, JT], F32, tag="wj")
            m16 = persist.tile([128, CT, H], DT, tag="m16")
            wv16 = persist.tile([128, CT, H], DT, tag="wv16")

            # ---- Phase 1: t-projection (fp16) + v-projection (fp16) ----
            with tc.tile_pool(name="xpool", bufs=3) as xpool, \
                 tc.tile_pool(name="pj", bufs=4, space="PSUM") as pj, \
                 tc.tile_pool(name="pv", bufs=2, space="PSUM") as pv:

                def load_xt(rb):
                    r0 = rb * RBS
                    t = xpool.tile([128, CT, RBS], DT, tag="xt",
                                   name=f"xt{rb}")
                    for ct in range(CT):
                        nc.sync.dma_start(out=t[:, ct, :],
                                          in_=xT16_d[:, ct, r0:r0 + RBS])
                    return t

                # DMA issue order = need order: interleave M c-tiles with the
                # first r-block's x so the ct=0 matmul's inputs arrive first.
                xts = [None] * RB
                xt0 = xpool.tile([128, CT, RBS], DT, tag="xt", name="xt0")
                for ct in range(CT):
                    nc.sync.dma_start(out=m16[:, ct, :], in_=m16_d[:, ct, :])
                    nc.sync.dma_start(out=xt0[:, ct, :],
                                      in_=xT16_d[:, ct, 0:RBS])
                xts[0] = xt0
                for ct in range(CT):
                    nc.sync.dma_start(out=wv16[:, ct, :], in_=wv16_d[:, ct, :])
                nc.sync.dma_start(out=wj, in_=wj_d)
                nc.sync.dma_start(out=xx8, in_=xx8_d)

                # PE warm-up: junk matmuls (no DMA deps) so the HAM clock-gate
                # reaches full rate while the first M/x DMAs are in flight.
                warm_l = xpool.tile([128, 128], DT, tag="warml", name="warml")
                warm_r = xpool.tile([128, 512], DT, tag="warmr", name="warmr")
                nc.vector.memset(warm_l, 0.0)
                nc.vector.memset(warm_r, 0.0)
                for i in range(16):
                    wp = pj.tile([128, RBS], F32, tag="pj", name=f"warm{i}")
                    nc.tensor.matmul(wp, warm_l, warm_r, start=True, stop=True)

                for rb in range(RB):
                    r0 = rb * RBS
                    if rb + 1 < RB:
                        xts[rb + 1] = load_xt(rb + 1)
                    xt = xts[rb]

                    # t-proj: tt8[:, ht, r0:r0+512] = 32*t^T  (fp8)
                    for ht in range(HT):
                        ps = pj.tile([128, RBS], F32, tag="pj")
                        for ct in range(CT):
                            nc.tensor.matmul(
                                ps,
                                m16[:, ct, ht * 128:(ht + 1) * 128],
                                xt[:, ct, :],
                                start=(ct == 0), stop=(ct == CT - 1),
                            )
                        if ht % 2 == 0:
                            nc.scalar.activation(
                                out=tt8[:, ht, r0:r0 + RBS], in_=ps,
                                func=mybir.ActivationFunctionType.Copy)
                        else:
                            nc.vector.tensor_copy(
                                out=tt8[:, ht, r0:r0 + RBS], in_=ps)

                    # v-proj for this r-block's 4 j-tiles (first half only)
                    if rb < KRB:
                        for j in range(RBS // 128):
                            jt = rb * (RBS // 128) + j
                            ps = pv.tile([128, H], F32, tag="pv")
                            for ct in range(CT):
                                xs = xt[:, ct, j * 128:(j + 1) * 128]
                                nc.tensor.matmul(
                                    ps[:, 0:512], xs, wv16[:, ct, 0:512],
                                    start=(ct == 0), stop=(ct == CT - 1))
                                nc.tensor.matmul(
                                    ps[:, 512:H], xs, wv16[:, ct, 512:H],
                                    start=(ct == 0), stop=(ct == CT - 1))
                            nc.vector.tensor_copy(out=vv[:, jt, :], in_=ps)

            # ---- Phase 2: attention (partial sums over this core's keys) ----
            with tc.tile_pool(name="opool", bufs=8) as opool, \
                 tc.tile_pool(name="spool", bufs=2) as spool, \
                 tc.tile_pool(name="ps_s", bufs=2, space="PSUM") as ps_s, \
                 tc.tile_pool(name="ps_o", bufs=6, space="PSUM") as ps_o:

                pending = []   # deferred work, flushed between PE groups

                def flush():
                    while pending:
                        pending.pop(0)()

                def emit_pv(og, jt, i0):
                    def go():
                        for ht in range(HT):
                            nc.tensor.matmul(
                                og[ht],
                                vv[:, jt, ht * 128:(ht + 1) * 128],
                                p16[:, jt, :],
                                start=(jt == 0), stop=(jt == JT - 1),
                            )
                    pending.append(go)

                def emit_den_and_evac(og, S16, ib, i0):
                    def go():
                        nc.sync.dma_start(out=den_d[ib], in_=S16)
                        for ht in range(HT):
                            ot = opool.tile([128, IBS], DT, tag="ot",
                                            name=f"ot{i0}_{ht}")
                            if ht % 2 == 0:
                                nc.vector.tensor_copy(out=ot, in_=og[ht])
                                dma = nc.sync.dma_start
                            else:
                                nc.scalar.activation(
                                    out=ot, in_=og[ht],
                                    func=mybir.ActivationFunctionType.Copy)
                                dma = nc.scalar.dma_start
                            dma(out=outT_d[ht * 128:(ht + 1) * 128,
                                           i0:i0 + IBS],
                                in_=ot)
                    pending.append(go)

                for ib in range(IB):
                    i0 = ib * IBS
                    og = [ps_o.tile([128, IBS], F32, tag="o", name=f"o{ib}_{g}")
                          for g in range(HT)]
                    Sf = spool.tile([128, IBS], F32, tag="Sf", name=f"Sf{ib}")
                    for jt in range(JT):
                        sps = ps_s.tile([128, IBS], F32, tag="s")
                        for t in range(CT // 2):
                            nc.tensor.matmul(
                                sps,
                                xx8[:, 2 * t:2 * t + 2, jt * 128:(jt + 1) * 128],
                                tt8[:, 2 * t:2 * t + 2, i0:i0 + IBS],
                                start=(t == 0), stop=(t == CT // 2 - 1),
                                perf_mode=DRMODE,
                            )
                        flush()
                        nc.scalar.activation(
                            out=p16[:, jt, :], in_=sps,
                            func=mybir.ActivationFunctionType.Exp,
                            scale=SCALE / TSCALE,
                            bias=wj[:, jt:jt + 1],
                        )
                        if jt == 0:
                            nc.vector.tensor_copy(out=Sf, in_=p16[:, jt, :])
                        else:
                            nc.vector.tensor_add(Sf, Sf, p16[:, jt, :])
                        if not (ib == IB - 1 and jt == JT - 1):
                            emit_pv(og, jt, i0)
                    S16 = spool.tile([128, IBS], DT, tag="S16", name=f"S16{ib}")
                    nc.vector.tensor_copy(out=S16, in_=Sf)
                    if ib < IB - 1:
                        emit_den_and_evac(og, S16, ib, i0)
                    else:
                        # eager epilogue: interleave the final j-tile's PV
                        # matmuls with per-h-tile evacuation so the output
                        # DMAs start as early as possible.
                        def epilogue(og=og, S16=S16, ib=ib, i0=i0):
                            nc.sync.dma_start(out=den_d[ib], in_=S16)
                            for ht in range(HT):
                                nc.tensor.matmul(
                                    og[ht],
                                    vv[:, JT - 1, ht * 128:(ht + 1) * 128],
                                    p16[:, JT - 1, :],
                                    start=False, stop=True,
                                )
                                ot = opool.tile([128, IBS], DT, tag="ot",
                                                name=f"ot{i0}_{ht}")
                                if ht % 2 == 0:
                                    nc.vector.tensor_copy(out=ot, in_=og[ht])
                                    dma = nc.sync.dma_start
                                else:
                                    nc.scalar.activation(
                                        out=ot, in_=og[ht],
                                        func=mybir.ActivationFunctionType.Copy)
                                    dma = nc.scalar.dma_start
                                dma(out=outT_d[ht * 128:(ht + 1) * 128,
                                               i0:i0 + IBS],
                                    in_=ot)
                        pending.append(epilogue)
                flush()
    nc.compile()
    return nc


@lru_cache(maxsize=1)
def _cached_program():
    return build_program()


def _ctile(a):
    """[C, X] -> [128, CT, X] (c-tile-structured, partition-major)."""
    return np.ascontiguousarray(
        a.reshape(CT, 128, a.shape[1]).transpose(1, 0, 2))


def _prep_in_maps(x, W_qkv, b_qkv):
    x = np.asarray(x, dtype=np.float32)
    W_qkv = np.asarray(W_qkv, dtype=np.float32)
    b_qkv = np.asarray(b_qkv, dtype=np.float32)
    Wq, Wk, Wv = W_qkv[:, :H], W_qkv[:, H:2 * H], W_qkv[:, 2 * H:]
    bq, bk = b_qkv[:H], b_qkv[H:2 * H]

    M32 = _ctile((TSCALE * (Wq @ Wk.T)).astype(np.float16))  # [128, CT, H]
    wv16 = _ctile(Wv.astype(np.float16))
    u = Wk @ bq                                              # [C]
    c0 = float(bq @ bk)

    in_maps = []
    for core in range(NCORES):
        b, kh = core // 2, core % 2
        xb = x[b]  # [N, C] f32
        if kh == 1:
            # Rotate so this core's key rows occupy rows [0, NK). Queries are
            # also rotated; the host rotates this core's outputs back.
            xb = np.concatenate([xb[NK:], xb[:NK]], axis=0)
        xT16 = _ctile(np.ascontiguousarray(xb.T).astype(np.float16))
        xx8 = _ctile(np.ascontiguousarray(xb[:NK].T).astype(NP8))
        wjv = ((xb[:NK] @ u + c0) * SCALE).astype(np.float32)
        wj = np.ascontiguousarray(wjv.reshape(JT, 128).T)    # [128, JT]
        in_maps.append({"xT16": xT16, "xx8": xx8, "m16": M32,
                       "wv16": wv16, "wj": wj})
    return in_maps


def _combine(results, b_qkv):
    bv = np.asarray(b_qkv, dtype=np.float32)[2 * H:]
    out = np.empty((B, N, C), dtype=np.float32)
    for b in range(B):
        o0 = results[2 * b]["outT"].astype(np.float32)       # [H, N]
        d0 = results[2 * b]["den"].astype(np.float32).sum(axis=1).reshape(N)
        o1 = results[2 * b + 1]["outT"].astype(np.float32)
        d1 = results[2 * b + 1]["den"].astype(np.float32).sum(axis=1).reshape(N)
        # core (2b+1) worked in rotated query order; rotate back
        o1 = np.concatenate([o1[:, NK:], o1[:, :NK]], axis=1)
        d1 = np.concatenate([d1[NK:], d1[:NK]])
        out[b] = ((o0 + o1) / (d0 + d1)).T + bv[None, :]
    return out


def kernel(x, W_qkv, b_qkv):
    nc = _cached_program()
    in_maps = _prep_in_maps(x, W_qkv, b_qkv)
    res = run_bass_kernel_spmd(nc, in_maps, core_ids=list(range(NCORES)))
    return _combine(res.results, b_qkv)


# revision 3
# speedup vs baseline: 1.1319x; 1.0033x over previous
"""Fused single-head attention (QKV proj + softmax(QK^T)V) on 8 trn2 cores.

Problem (hardcoded): x [4, 4096, 768] f32, W_qkv [768, 2304] f32, b_qkv
[2304] f32:
  qkv = x @ W_qkv + b_qkv ; q,k,v = split(qkv, 3)
  out = softmax(q k^T / sqrt(768)) v          -> [4, 4096, 768] f32

Sharding: batch (4) x key-halves (2) -> 8 cores, no cross-core traffic.
Each core computes PARTIAL attention sums over its 2048 keys; the host
combines pairs: (o0 + o1) / (d0 + d1). No max-subtraction (scores are
O(1)).

v2 math (vs the fp16 v1): the score chain runs through fp8-e4m3
DoubleRow matmuls at 2x PE rate:
  - k-projection is eliminated: scores = x M x^T * SCALE with
    M = W_q W_k^T precomputed on host (fp16, pre-scaled by 32).
  - q-bias cancels in softmax; k-bias folds into a per-key additive
    score bias w_j = (x @ (W_k b_q) + b_q.b_k) * SCALE computed on the
    host and applied inside the exp activation (per-partition bias).
  - v-bias is added by the host after the combine.
  - t = x @ (32 M) computed on-device in fp16 (f32 accum), stored fp8.
  - scores psum = (32 t) @ x8^T via 3 fp8 DoubleRow matmuls (256-deep
    contraction each); exp scale folds the 1/32 and 1/sqrt(H).
  - PV and v-projection stay fp16: fp8 there fails the 2e-2 gate
    (v/p quantization error passes straight to the output).
Measured-on-hw notes: DoubleRow sustains full rate ONLY when both
operands are sliced from single big SBUF tiles (separate small tiles
rotating per-matmul serialize ~2.3x); f32->fp8 evac on ScalarE/DVE is
exact round-to-nearest (required: truncation would double the error).

Layouts (all per-partition-major, single big tiles):
  xx8 [128, 6, 2048] fp8   x^T for this core's keys, c-tile-structured
  tt8 [128, 6, 4096] fp8   (32 t)^T, written by t-proj evac
  vv  [128, 16, 768] fp16  v tiles (j-tile-structured)
  p16 [128, 16, 512] fp16  exp(scores) for the current i-block
PSUM: scores 2 banks + out accumulators 6 banks.
"""

import math
from contextlib import ExitStack
from functools import lru_cache

import ml_dtypes
import numpy as np

import concourse.bacc as bacc
import concourse.bass as bass
import concourse.tile as tile
from concourse import mybir
from concourse.bass_utils import run_bass_kernel_spmd

B, N, C = 4, 4096, 768
H = 768
NCORES = 8
NK = N // 2      # keys per core
DT = mybir.dt.float16
DT8 = mybir.dt.float8e4
F32 = mybir.dt.float32
SCALE = 1.0 / math.sqrt(H)
TSCALE = 32.0    # host pre-scales M by this; exp folds 1/TSCALE back out
DRMODE = mybir.MatmulPerfMode.DoubleRow

CT = C // 128    # 6 contraction tiles
HT = H // 128    # 6 head tiles
JT = NK // 128   # 16 key tiles per core
RB = 8           # r-blocks of 512 over the 4096 rows
RBS = N // RB    # 512
KRB = RB // 2    # r-blocks containing this core's keys (first 4)
IB = 8
IBS = N // IB    # 512
NP8 = ml_dtypes.float8_e4m3


def build_program():
    nc = bacc.Bacc(
        "TRN2",
        target_bir_lowering=False,
        debug=False,
        enable_asserts=False,
        num_devices=NCORES,
    )
    xT16_d = nc.dram_tensor("xT16", [128, CT, N], DT, kind="ExternalInput").ap()
    xx8_d = nc.dram_tensor("xx8", [128, CT, NK], DT8, kind="ExternalInput").ap()
    m16_d = nc.dram_tensor("m16", [128, CT, H], DT, kind="ExternalInput").ap()
    wv16_d = nc.dram_tensor("wv16", [128, CT, H], DT, kind="ExternalInput").ap()
    wj_d = nc.dram_tensor("wj", [128, JT], F32, kind="ExternalInput").ap()
    outT_d = nc.dram_tensor("outT", [H, N], DT, kind="ExternalOutput").ap()
    # per-partition partial softmax denominators; host sums over axis 1
    den_d = nc.dram_tensor("den", [IB, 128, IBS], DT, kind="ExternalOutput").ap()

    with tile.TileContext(nc) as tc:
        with ExitStack() as ctx:
            persist = ctx.enter_context(tc.tile_pool(name="persist", bufs=1))

            xx8 = persist.tile([128, CT, NK], DT8, tag="xx8")
            tt8 = persist.tile([128, CT, N], DT8, tag="tt8")
            vv = persist.tile([128, JT, H], DT, tag="vv")
            p16 = persist.tile([128, JT, IBS], DT, tag="p16")
            wj = persist.tile([128, JT], F32, tag="wj")
            m16 = persist.tile([128, CT, H], DT, tag="m16")
            wv16 = persist.tile([128, CT, H], DT, tag="wv16")

            # ---- Phase 1: t-projection (fp16) + v-projection (fp16) ----
            with tc.tile_pool(name="xpool", bufs=3) as xpool, \
                 tc.tile_pool(name="pj", bufs=4, space="PSUM") as pj, \
                 tc.tile_pool(name="pv", bufs=2, space="PSUM") as pv:

                def load_xt(rb):
                    r0 = rb * RBS
                    t = xpool.tile([128, CT, RBS], DT, tag="xt",
                                   name=f"xt{rb}")
                    for ct in range(CT):
                        nc.sync.dma_start(out=t[:, ct, :],
                                          in_=xT16_d[:, ct, r0:r0 + RBS])
                    return t

                # DMA issue order = need order: interleave M c-tiles with the
                # first r-block's x so the ct=0 matmul's inputs arrive first.
                xts = [None] * RB
                xt0 = xpool.tile([128, CT, RBS], DT, tag="xt", name="xt0")
                for ct in range(CT):
                    nc.sync.dma_start(out=m16[:, ct, :], in_=m16_d[:, ct, :])
                    nc.sync.dma_start(out=xt0[:, ct, :],
                                      in_=xT16_d[:, ct, 0:RBS])
                xts[0] = xt0
                for ct in range(CT):
                    nc.sync.dma_start(out=wv16[:, ct, :], in_=wv16_d[:, ct, :])
                nc.sync.dma_start(out=wj, in_=wj_d)
                nc.sync.dma_start(out=xx8, in_=xx8_d)

                # PE warm-up: junk matmuls (no DMA deps) so the HAM clock-gate
                # reaches full rate while the first M/x DMAs are in flight.
                warm_l = xpool.tile([128, 128], DT, tag="warml", name="warml")
                warm_r = xpool.tile([128, 512], DT, tag="warmr", name="warmr")
                nc.vector.memset(warm_l, 0.0)
                nc.vector.memset(warm_r, 0.0)
                for i in range(16):
                    wp = pj.tile([128, RBS], F32, tag="pj", name=f"warm{i}")
                    nc.tensor.matmul(wp, warm_l, warm_r, start=True, stop=True)

                for rb in range(RB):
                    r0 = rb * RBS
                    if rb + 1 < RB:
                        xts[rb + 1] = load_xt(rb + 1)
                    xt = xts[rb]

                    # t-proj: tt8[:, ht, r0:r0+512] = 32*t^T  (fp8)
                    for ht in range(HT):
                        ps = pj.tile([128, RBS], F32, tag="pj")
                        for ct in range(CT):
                            nc.tensor.matmul(
                                ps,
                                m16[:, ct, ht * 128:(ht + 1) * 128],
                                xt[:, ct, :],
                                start=(ct == 0), stop=(ct == CT - 1),
                            )
                        if ht % 2 == 0:
                            nc.scalar.activation(
                                out=tt8[:, ht, r0:r0 + RBS], in_=ps,
                                func=mybir.ActivationFunctionType.Copy)
                        else:
                            nc.vector.tensor_copy(
                                out=tt8[:, ht, r0:r0 + RBS], in_=ps)

                    # v-proj for this r-block's 4 j-tiles (first half only)
                    if rb < KRB:
                        for j in range(RBS // 128):
                            jt = rb * (RBS // 128) + j
                            ps = pv.tile([128, H], F32, tag="pv")
                            for ct in range(CT):
                                xs = xt[:, ct, j * 128:(j + 1) * 128]
                                nc.tensor.matmul(
                                    ps[:, 0:512], xs, wv16[:, ct, 0:512],
                                    start=(ct == 0), stop=(ct == CT - 1))
                                nc.tensor.matmul(
                                    ps[:, 512:H], xs, wv16[:, ct, 512:H],
                                    start=(ct == 0), stop=(ct == CT - 1))
                            nc.vector.tensor_copy(out=vv[:, jt, :], in_=ps)

            # ---- Phase 2: attention (partial sums over this core's keys) ----
            with tc.tile_pool(name="opool", bufs=8) as opool, \
                 tc.tile_pool(name="spool", bufs=2) as spool, \
                 tc.tile_pool(name="ps_s", bufs=2, space="PSUM") as ps_s, \
                 tc.tile_pool(name="ps_o", bufs=6, space="PSUM") as ps_o:

                pending = []   # deferred work, flushed between PE groups

                def flush():
                    while pending:
                        pending.pop(0)()

                def emit_pv(og, jt, i0):
                    def go():
                        for ht in range(HT):
                            nc.tensor.matmul(
                                og[ht],
                                vv[:, jt, ht * 128:(ht + 1) * 128],
                                p16[:, jt, :],
                                start=(jt == 0), stop=False,
                            )
                    pending.append(go)

                def emit_epilogue(og, S16, ib, i0):
                    # final j-tile's PV matmuls interleaved with per-h-tile
                    # evacuation (all on DVE; ScalarE stays dedicated to exp)
                    # so og banks free early for the next i-block.
                    def go():
                        nc.sync.dma_start(out=den_d[ib], in_=S16)
                        for ht in range(HT):
                            nc.tensor.matmul(
                                og[ht],
                                vv[:, JT - 1, ht * 128:(ht + 1) * 128],
                                p16[:, JT - 1, :],
                                start=False, stop=True,
                            )
                            ot = opool.tile([128, IBS], DT, tag="ot",
                                            name=f"ot{i0}_{ht}")
                            nc.vector.tensor_copy(out=ot, in_=og[ht])
                            nc.sync.dma_start(
                                out=outT_d[ht * 128:(ht + 1) * 128,
                                           i0:i0 + IBS],
                                in_=ot)
                    pending.append(go)

                for ib in range(IB):
                    i0 = ib * IBS
                    og = [ps_o.tile([128, IBS], F32, tag="o", name=f"o{ib}_{g}")
                          for g in range(HT)]
                    Sf = spool.tile([128, IBS], F32, tag="Sf", name=f"Sf{ib}")
                    for jt in range(JT):
                        sps = ps_s.tile([128, IBS], F32, tag="s")
                        for t in range(CT // 2):
                            nc.tensor.matmul(
                                sps,
                                xx8[:, 2 * t:2 * t + 2, jt * 128:(jt + 1) * 128],
                                tt8[:, 2 * t:2 * t + 2, i0:i0 + IBS],
                                start=(t == 0), stop=(t == CT // 2 - 1),
                                perf_mode=DRMODE,
                            )
                        flush()
                        nc.scalar.activation(
                            out=p16[:, jt, :], in_=sps,
                            func=mybir.ActivationFunctionType.Exp,
                            scale=SCALE / TSCALE,
                            bias=wj[:, jt:jt + 1],
                        )
                        if jt == 0:
                            nc.vector.tensor_copy(out=Sf, in_=p16[:, jt, :])
                        else:
                            nc.vector.tensor_add(Sf, Sf, p16[:, jt, :])
                        if jt < JT - 1:
                            emit_pv(og, jt, i0)
                        else:
                            S16 = spool.tile([128, IBS], DT, tag="S16",
                                             name=f"S16{ib}")
                            nc.vector.tensor_copy(out=S16, in_=Sf)
                            emit_epilogue(og, S16, ib, i0)
                flush()
    nc.compile()
    return nc


@lru_cache(maxsize=1)
def _cached_program():
    return build_program()


def _ctile(a):
    """[C, X] -> [128, CT, X] (c-tile-structured, partition-major)."""
    return np.ascontiguousarray(
        a.reshape(CT, 128, a.shape[1]).transpose(1, 0, 2))


def _prep_in_maps(x, W_qkv, b_qkv):
    x = np.asarray(x, dtype=np.float32)
    W_qkv = np.asarray(W_qkv, dtype=np.float32)
    b_qkv = np.asarray(b_qkv, dtype=np.float32)
    Wq, Wk, Wv = W_qkv[:, :H], W_qkv[:, H:2 * H], W_qkv[:, 2 * H:]
    bq, bk = b_qkv[:H], b_qkv[H:2 * H]

    M32 = _ctile((TSCALE * (Wq @ Wk.T)).astype(np.float16))  # [128, CT, H]
    wv16 = _ctile(Wv.astype(np.float16))
    u = Wk @ bq                                              # [C]
    c0 = float(bq @ bk)

    in_maps = []
    for core in range(NCORES):
        b, kh = core // 2, core % 2
        xb = x[b]  # [N, C] f32
        if kh == 1:
            # Rotate so this core's key rows occupy rows [0, NK). Queries are
            # also rotated; the host rotates this core's outputs back.
            xb = np.concatenate([xb[NK:], xb[:NK]], axis=0)
        xT16 = _ctile(np.ascontiguousarray(xb.T).astype(np.float16))
        xx8 = _ctile(np.ascontiguousarray(xb[:NK].T).astype(NP8))
        wjv = ((xb[:NK] @ u + c0) * SCALE).astype(np.float32)
        wj = np.ascontiguousarray(wjv.reshape(JT, 128).T)    # [128, JT]
        in_maps.append({"xT16": xT16, "xx8": xx8, "m16": M32,
                       "wv16": wv16, "wj": wj})
    return in_maps


def _combine(results, b_qkv):
    bv = np.asarray(b_qkv, dtype=np.float32)[2 * H:]
    out = np.empty((B, N, C), dtype=np.float32)
    for b in range(B):
        o0 = results[2 * b]["outT"].astype(np.float32)       # [H, N]
        d0 = results[2 * b]["den"].astype(np.float32).sum(axis=1).reshape(N)
        o1 = results[2 * b + 1]["outT"].astype(np.float32)
        d1 = results[2 * b + 1]["den"].astype(np.float32).sum(axis=1).reshape(N)
        # core (2b+1) worked in rotated query order; rotate back
        o1 = np.concatenate([o1[:, NK:], o1[:, :NK]], axis=1)
        d1 = np.concatenate([d1[NK:], d1[:NK]])
        out[b] = ((o0 + o1) / (d0 + d1)).T + bv[None, :]
    return out


def kernel(x, W_qkv, b_qkv):
    nc = _cached_program()
    in_maps = _prep_in_maps(x, W_qkv, b_qkv)
    res = run_bass_kernel_spmd(nc, in_maps, core_ids=list(range(NCORES)))
    return _combine(res.results, b_qkv)


# revision 5
# speedup vs baseline: 1.3591x; 1.2008x over previous
"""Fused single-head attention (QKV proj + softmax(QK^T)V) on 8 trn2 cores.

Problem (hardcoded): x [4, 4096, 768] f32, W_qkv [768, 2304] f32, b_qkv
[2304] f32:
  qkv = x @ W_qkv + b_qkv ; q,k,v = split(qkv, 3)
  out = softmax(q k^T / sqrt(768)) v          -> [4, 4096, 768] f32

Sharding: batch (4) x key-halves (2) -> 8 cores, no cross-core traffic.
Each core computes PARTIAL attention sums over its 2048 keys; the host
combines pairs: (o0 + o1) / (d0 + d1). No max-subtraction (scores are
O(1)).

v2 math (vs the fp16 v1): the score chain runs through fp8-e4m3
DoubleRow matmuls at 2x PE rate:
  - k-projection is eliminated: scores = x M x^T * SCALE with
    M = W_q W_k^T precomputed on host (fp16, pre-scaled by 32).
  - q-bias cancels in softmax; k-bias folds into a per-key additive
    score bias w_j = (x @ (W_k b_q) + b_q.b_k) * SCALE computed on the
    host and applied inside the exp activation (per-partition bias).
  - v-bias is added by the host after the combine.
  - t = x @ (32 M) computed on-device in fp16 (f32 accum), stored fp8.
  - scores psum = (32 t) @ x8^T via 3 fp8 DoubleRow matmuls (256-deep
    contraction each); exp scale folds the 1/32 and 1/sqrt(H).
  - PV and v-projection stay fp16: fp8 there fails the 2e-2 gate
    (v/p quantization error passes straight to the output).
Measured-on-hw notes: DoubleRow sustains full rate ONLY when both
operands are sliced from single big SBUF tiles (separate small tiles
rotating per-matmul serialize ~2.3x); f32->fp8 evac on ScalarE/DVE is
exact round-to-nearest (required: truncation would double the error).

Layouts (all per-partition-major, single big tiles):
  xx8 [128, 6, 2048] fp8   x^T for this core's keys, c-tile-structured
  tt8 [128, 6, 4096] fp8   (32 t)^T, written by t-proj evac
  vv  [128, 16, 768] fp16  v tiles (j-tile-structured)
  p16 [128, 16, 512] fp16  exp(scores) for the current i-block
PSUM: scores 2 banks + out accumulators 6 banks.
"""

import math
from contextlib import ExitStack
from functools import lru_cache

import ml_dtypes
import numpy as np

import concourse.bacc as bacc
import concourse.bass as bass
import concourse.tile as tile
from concourse import mybir
from concourse.bass_utils import run_bass_kernel_spmd

B, N, C = 4, 4096, 768
H = 768
NCORES = 8
NK = N // 2      # keys per core
DT = mybir.dt.float16
DT8 = mybir.dt.float8e4
F32 = mybir.dt.float32
SCALE = 1.0 / math.sqrt(H)
TSCALE = 32.0    # host pre-scales M by this; exp folds 1/TSCALE back out
DRMODE = mybir.MatmulPerfMode.DoubleRow

CT = C // 128    # 6 contraction tiles
HT = H // 128    # 6 head tiles
JT = NK // 128   # 16 key tiles per core
RB = 8           # r-blocks of 512 over the 4096 rows
RBS = N // RB    # 512
KRB = RB // 2    # r-blocks containing this core's keys (first 4)
IB = 8
IBS = N // IB    # 512
NP8 = ml_dtypes.float8_e4m3


def build_program():
    nc = bacc.Bacc(
        "TRN2",
        target_bir_lowering=False,
        debug=False,
        enable_asserts=False,
        num_devices=NCORES,
    )
    xT16_d = nc.dram_tensor("xT16", [128, CT, N], DT, kind="ExternalInput").ap()
    xx8_d = nc.dram_tensor("xx8", [128, CT, NK], DT8, kind="ExternalInput").ap()
    m16_d = nc.dram_tensor("m16", [128, CT, H], DT, kind="ExternalInput").ap()
    wv16_d = nc.dram_tensor("wv16", [128, CT, H], DT, kind="ExternalInput").ap()
    wj_d = nc.dram_tensor("wj", [128, JT], F32, kind="ExternalInput").ap()
    outT_d = nc.dram_tensor("outT", [H, N], DT, kind="ExternalOutput").ap()
    # per-partition partial softmax denominators; host sums over axis 1
    den_d = nc.dram_tensor("den", [IB, 128, IBS], DT, kind="ExternalOutput").ap()

    with tile.TileContext(nc) as tc:
        with ExitStack() as ctx:
            persist = ctx.enter_context(tc.tile_pool(name="persist", bufs=1))

            xx8 = persist.tile([128, CT, NK], DT8, tag="xx8")
            tt8 = persist.tile([128, CT, N], DT8, tag="tt8")
            vv = persist.tile([128, JT, H], DT, tag="vv")
            p16 = persist.tile([128, JT, IBS], DT, tag="p16")
            wj = persist.tile([128, JT], F32, tag="wj")
            m16 = persist.tile([128, CT, H], DT, tag="m16")
            wv16 = persist.tile([128, CT, H], DT, tag="wv16")

            # ---- Phase 1: t-projection (fp16) + v-projection (fp16) ----
            with tc.tile_pool(name="xpool", bufs=3) as xpool, \
                 tc.tile_pool(name="pj", bufs=4, space="PSUM") as pj, \
                 tc.tile_pool(name="pv", bufs=2, space="PSUM") as pv:

                def load_xt(rb):
                    r0 = rb * RBS
                    t = xpool.tile([128, CT, RBS], DT, tag="xt",
                                   name=f"xt{rb}")
                    for ct in range(CT):
                        nc.sync.dma_start(out=t[:, ct, :],
                                          in_=xT16_d[:, ct, r0:r0 + RBS])
                    return t

                # DMA issue order = need order: interleave M c-tiles with the
                # first r-block's x so the ct=0 matmul's inputs arrive first.
                xts = [None] * RB
                xt0 = xpool.tile([128, CT, RBS], DT, tag="xt", name="xt0")
                for ct in range(CT):
                    nc.sync.dma_start(out=m16[:, ct, :], in_=m16_d[:, ct, :])
                    nc.sync.dma_start(out=xt0[:, ct, :],
                                      in_=xT16_d[:, ct, 0:RBS])
                xts[0] = xt0
                for ct in range(CT):
                    nc.sync.dma_start(out=wv16[:, ct, :], in_=wv16_d[:, ct, :])
                nc.sync.dma_start(out=wj, in_=wj_d)
                nc.sync.dma_start(out=xx8, in_=xx8_d)

                # PE warm-up: junk matmuls (no DMA deps) so the HAM clock-gate
                # reaches full rate while the first M/x DMAs are in flight.
                warm_l = xpool.tile([128, 128], DT, tag="warml", name="warml")
                warm_r = xpool.tile([128, 512], DT, tag="warmr", name="warmr")
                nc.vector.memset(warm_l, 0.0)
                nc.vector.memset(warm_r, 0.0)
                for i in range(16):
                    wp = pj.tile([128, RBS], F32, tag="pj", name=f"warm{i}")
                    nc.tensor.matmul(wp, warm_l, warm_r, start=True, stop=True)

                for rb in range(RB):
                    r0 = rb * RBS
                    if rb + 1 < RB:
                        xts[rb + 1] = load_xt(rb + 1)
                    xt = xts[rb]

                    # t-proj: tt8[:, ht, r0:r0+512] = 32*t^T  (fp8)
                    for ht in range(HT):
                        ps = pj.tile([128, RBS], F32, tag="pj")
                        for ct in range(CT):
                            nc.tensor.matmul(
                                ps,
                                m16[:, ct, ht * 128:(ht + 1) * 128],
                                xt[:, ct, :],
                                start=(ct == 0), stop=(ct == CT - 1),
                            )
                        if ht % 2 == 0:
                            nc.scalar.activation(
                                out=tt8[:, ht, r0:r0 + RBS], in_=ps,
                                func=mybir.ActivationFunctionType.Copy)
                        else:
                            nc.vector.tensor_copy(
                                out=tt8[:, ht, r0:r0 + RBS], in_=ps)

                    # v-proj for this r-block's 4 j-tiles (first half only)
                    if rb < KRB:
                        for j in range(RBS // 128):
                            jt = rb * (RBS // 128) + j
                            ps = pv.tile([128, H], F32, tag="pv")
                            for ct in range(CT):
                                xs = xt[:, ct, j * 128:(j + 1) * 128]
                                nc.tensor.matmul(
                                    ps[:, 0:512], xs, wv16[:, ct, 0:512],
                                    start=(ct == 0), stop=(ct == CT - 1))
                                nc.tensor.matmul(
                                    ps[:, 512:H], xs, wv16[:, ct, 512:H],
                                    start=(ct == 0), stop=(ct == CT - 1))
                            nc.vector.tensor_copy(out=vv[:, jt, :], in_=ps)

            # ---- Phase 2: attention (partial sums over this core's keys) ----
            # Per i-block, two passes over the 16 j-tiles:
            #   pass A: scores (3 DR mms) + exp + PV for h-tiles 0-2,
            #           with PV-A running TWO score-groups behind so the
            #           823ns exp latency hides under ~1.9us of PE work.
            #   pass B: PV for h-tiles 3-5 from the stored p16 (no deps).
            # PSUM: scores 3 banks (tag "s") + og-A 3 (tag "oa") = 6; pass B's
            # og-B reuses the "s" slots (same size) once scores are drained.
            with tc.tile_pool(name="opool", bufs=8) as opool, \
                 tc.tile_pool(name="spool", bufs=2) as spool, \
                 tc.tile_pool(name="psum2", bufs=1, space="PSUM") as psum2:

                HA = HT // 2     # h-tiles in pass A
                pending = []     # deferred PE work

                def emit_pva(oga, jt):
                    def go():
                        for ht in range(HA):
                            nc.tensor.matmul(
                                oga[ht],
                                vv[:, jt, ht * 128:(ht + 1) * 128],
                                p16[:, jt, :],
                                start=(jt == 0), stop=(jt == JT - 1),
                            )
                    pending.append(go)

                for ib in range(IB):
                    i0 = ib * IBS
                    oga = [psum2.tile([128, IBS], F32, tag="oa", bufs=3,
                                      name=f"oa{ib}_{g}") for g in range(HA)]
                    Sf = spool.tile([128, IBS], F32, tag="Sf", name=f"Sf{ib}")
                    # ---- pass A ----
                    for jt in range(JT):
                        sps = psum2.tile([128, IBS], F32, tag="s", bufs=3)
                        for t in range(CT // 2):
                            nc.tensor.matmul(
                                sps,
                                xx8[:, 2 * t:2 * t + 2, jt * 128:(jt + 1) * 128],
                                tt8[:, 2 * t:2 * t + 2, i0:i0 + IBS],
                                start=(t == 0), stop=(t == CT // 2 - 1),
                                perf_mode=DRMODE,
                            )
                        while len(pending) > 1:
                            pending.pop(0)()
                        nc.scalar.activation(
                            out=p16[:, jt, :], in_=sps,
                            func=mybir.ActivationFunctionType.Exp,
                            scale=SCALE / TSCALE,
                            bias=wj[:, jt:jt + 1],
                        )
                        if jt == 0:
                            nc.vector.tensor_copy(out=Sf, in_=p16[:, jt, :])
                        else:
                            nc.vector.tensor_add(Sf, Sf, p16[:, jt, :])
                        emit_pva(oga, jt)
                    while pending:
                        pending.pop(0)()
                    S16 = spool.tile([128, IBS], DT, tag="S16", name=f"S16{ib}")
                    nc.vector.tensor_copy(out=S16, in_=Sf)
                    nc.sync.dma_start(out=den_d[ib], in_=S16)
                    # og-A evac on DVE (runs while PE does pass B)
                    for ht in range(HA):
                        ot = opool.tile([128, IBS], DT, tag="ot",
                                        name=f"ota{i0}_{ht}")
                        nc.vector.tensor_copy(out=ot, in_=oga[ht])
                        nc.sync.dma_start(
                            out=outT_d[ht * 128:(ht + 1) * 128, i0:i0 + IBS],
                            in_=ot)
                    # ---- pass B (g outer: each og-B bank drains while the
                    # next accumulates, so evacs hide under PE work) ----
                    ogb = [psum2.tile([128, IBS], F32, tag="s", bufs=3,
                                      name=f"ob{ib}_{g}") for g in range(HA)]
                    for g in range(HA):
                        ht = HA + g
                        for jt in range(JT):
                            nc.tensor.matmul(
                                ogb[g],
                                vv[:, jt, ht * 128:(ht + 1) * 128],
                                p16[:, jt, :],
                                start=(jt == 0), stop=(jt == JT - 1),
                            )
                        ot = opool.tile([128, IBS], DT, tag="ot",
                                        name=f"otb{i0}_{g}")
                        nc.vector.tensor_copy(out=ot, in_=ogb[g])
                        nc.sync.dma_start(
                            out=outT_d[ht * 128:(ht + 1) * 128, i0:i0 + IBS],
                            in_=ot)
    nc.compile()
    return nc


@lru_cache(maxsize=1)
def _cached_program():
    return build_program()


def _ctile(a):
    """[C, X] -> [128, CT, X] (c-tile-structured, partition-major)."""
    return np.ascontiguousarray(
        a.reshape(CT, 128, a.shape[1]).transpose(1, 0, 2))


def _prep_in_maps(x, W_qkv, b_qkv):
    x = np.asarray(x, dtype=np.float32)
    W_qkv = np.asarray(W_qkv, dtype=np.float32)
    b_qkv = np.asarray(b_qkv, dtype=np.float32)
    Wq, Wk, Wv = W_qkv[:, :H], W_qkv[:, H:2 * H], W_qkv[:, 2 * H:]
    bq, bk = b_qkv[:H], b_qkv[H:2 * H]

    M32 = _ctile((TSCALE * (Wq @ Wk.T)).astype(np.float16))  # [128, CT, H]
    wv16 = _ctile(Wv.astype(np.float16))
    u = Wk @ bq                                              # [C]
    c0 = float(bq @ bk)

    in_maps = []
    for core in range(NCORES):
        b, kh = core // 2, core % 2
        xb = x[b]  # [N, C] f32
        if kh == 1:
            # Rotate so this core's key rows occupy rows [0, NK). Queries are
            # also rotated; the host rotates this core's outputs back.
            xb = np.concatenate([xb[NK:], xb[:NK]], axis=0)
        xT16 = _ctile(np.ascontiguousarray(xb.T).astype(np.float16))
        xx8 = _ctile(np.ascontiguousarray(xb[:NK].T).astype(NP8))
        wjv = ((xb[:NK] @ u + c0) * SCALE).astype(np.float32)
        wj = np.ascontiguousarray(wjv.reshape(JT, 128).T)    # [128, JT]
        in_maps.append({"xT16": xT16, "xx8": xx8, "m16": M32,
                       "wv16": wv16, "wj": wj})
    return in_maps


def _combine(results, b_qkv):
    bv = np.asarray(b_qkv, dtype=np.float32)[2 * H:]
    out = np.empty((B, N, C), dtype=np.float32)
    for b in range(B):
        o0 = results[2 * b]["outT"].astype(np.float32)       # [H, N]
        d0 = results[2 * b]["den"].astype(np.float32).sum(axis=1).reshape(N)
        o1 = results[2 * b + 1]["outT"].astype(np.float32)
        d1 = results[2 * b + 1]["den"].astype(np.float32).sum(axis=1).reshape(N)
        # core (2b+1) worked in rotated query order; rotate back
        o1 = np.concatenate([o1[:, NK:], o1[:, :NK]], axis=1)
        d1 = np.concatenate([d1[NK:], d1[:NK]])
        out[b] = ((o0 + o1) / (d0 + d1)).T + bv[None, :]
    return out


def kernel(x, W_qkv, b_qkv):
    nc = _cached_program()
    in_maps = _prep_in_maps(x, W_qkv, b_qkv)
    res = run_bass_kernel_spmd(nc, in_maps, core_ids=list(range(NCORES)))
    return _combine(res.results, b_qkv)
